# revision 32
# baseline (speedup 1.0000x reference)
"""Single attention head (B=8, S=2048, D_IN=1024, D_OUT=64) on 8 TRN2 NeuronCores.

Strategy: pure data-parallel over batch -- core b computes batch element b's
full attention head. No collectives.

Per-core dataflow, one shared 8-bank PSUM pool so all phases overlap:
  - K/Q projections run in FP8 with DoubleRow packing (2 contraction rows per
    PE cell -> 256-deep contraction per pass, 0.5 cyc/row): the host supplies
    seq and the stacked [wk|wq] weights (x32 so fp8 stays in normal range; the
    x1024 on scores is folded into the exp scale) in the [128, 2, *]
    row-interleaved layout as a free reshape.  The fp8 seq copy is only 2 MB,
    so it loads first and score work starts ~10 us before the bf16 seq lands.
    One 256-row-deep matmul per contraction chunk yields kT on psum partitions
    0:64 and qT on 64:128; one VectorE drain adds the stacked x32[bk; bq] bias
    (bf16 out).  Numerics: fp8 q/k projections measure 3.2e-3 overall rel err
    vs the f32 reference (gate is 2e-2); V must stay bf16 (fp8 V -> 2.05e-2).
  - V projection from the bf16 seq copy; vT re-transposed on the TensorEngine
    into natural [k, 65] layout; masked-out keys get their V rows (and the
    appended ones-column) ZEROED, applying the attention mask exactly (masked
    keys contribute nothing to the context sum or the softmax denominator).
  - scores: key-chunk PAIRS row-tiled on the PE (chunk A on array rows 0:64,
    B on 64:128 -- disjoint row groups run concurrently) into one [128, 1024]
    psum pair-tile; ONE exp activation covers the pair (no max-subtraction:
    |scores/sqrt(S)| << 1).  Context matmuls (bf16) accumulate ctxT[65, q],
    row 64 = keep-mask dot exp = softmax denominator; they are emitted on a
    deferred queue so the strict-FIFO PE never idles waiting for an exp, and
    the 16-deep exp backlog built while the bf16 seq is still loading keeps
    ScalarE saturated through the V phase.
  - finalize per q-chunk: transpose ctxT back to [q, 65], multiply rows by
    1/ctx[.., 64], collect into out_sb, per-q-chunk DMA out.
"""

import numpy as np
import ml_dtypes

import concourse.bass as bass  # noqa: F401  (bass types used via tile/bacc)
import concourse.mybir as mybir
import concourse.tile as tile
from concourse import bacc
from concourse.bass_utils import run_bass_kernel_spmd

B, S, D, F = 8, 2048, 1024, 64
NCORES = 8
BF = mybir.dt.bfloat16
F8 = mybir.dt.float8e4
F32 = mybir.dt.float32
# reference scales by sqrt(S); q and k each carry x32 from the fp8 weight scaling
SCALE = 1.0 / (1024.0 * float(np.sqrt(np.float32(S))))
SC = 512  # matmul moving free-dim
KCH = S // 128  # 16 key chunks
DCH = D // 128  # 8 bf16 contraction chunks
DR = D // 256  # 4 fp8 DoubleRow contraction chunks


def _emit(nc):
    seqf8_d = nc.declare_dram_parameter("seqf8", [D, S], F8, isOutput=False)
    seqb_d = nc.declare_dram_parameter("seqb", [D, S], BF, isOutput=False)
    # stacked x32*[wk | wq] and x32*[wq | wk] fp8
    wkq_d = nc.declare_dram_parameter("wkq", [D, 128], F8, isOutput=False)
    wqk_d = nc.declare_dram_parameter("wqk", [D, 128], F8, isOutput=False)
    wv_d = nc.declare_dram_parameter("wv", [D, F], BF, isOutput=False)
    # misc f32 [128, 19]: col0 = 32*[bk; bq] stacked, col2 rows0:64 = bv,
    # cols 3:19 = 0/1 keep-mask per key chunk [128, 16]
    misc_d = nc.declare_dram_parameter("misc", [128, 3 + KCH], F32, isOutput=False)
    identb_d = nc.declare_dram_parameter("identb", [128, 128], BF, isOutput=False)
    identf_d = nc.declare_dram_parameter("identf", [128, 128], F32, isOutput=False)
    out_d = nc.declare_dram_parameter("out", [S, F], F32, isOutput=True)

    with tile.TileContext(nc) as tc:
        _body(nc, tc, seqf8_d, seqb_d, wkq_d, wqk_d, wv_d, misc_d, identb_d, identf_d, out_d)
    nc.compile()


def _body(nc, tc, seqf8_d, seqb_d, wkq_d, wqk_d, wv_d, misc_d, identb_d, identf_d, out_d):
    from contextlib import ExitStack

    with ExitStack() as ctx:
        const = ctx.enter_context(tc.tile_pool(name="const", bufs=1))
        big = ctx.enter_context(tc.tile_pool(name="big", bufs=1))
        sbw = ctx.enter_context(tc.tile_pool(name="sbw", bufs=1))
        ps = ctx.enter_context(tc.tile_pool(name="ps", space="PSUM", bufs=1))

        # ---- constant loads (scalar-engine HWDGE queue) ----
        misc_sb = const.tile([128, 3 + KCH], F32, name="misc_sb")
        nc.scalar.dma_start(out=misc_sb[:], in_=misc_d.ap())
        # DoubleRow layouts: row index d = 256c + 2p + i -> [p, c, i, f]
        wkq_sb = const.tile([128, DR, 2, 128], F8, name="wkq_sb")
        nc.scalar.dma_start(
            out=wkq_sb[:], in_=wkq_d.ap().rearrange("(c p i) f -> p c i f", p=128, i=2)
        )
        wqk_sb = const.tile([128, DR, 2, 128], F8, name="wqk_sb")
        nc.scalar.dma_start(
            out=wqk_sb[:], in_=wqk_d.ap().rearrange("(c p i) f -> p c i f", p=128, i=2)
        )
        # wv/identb/identf are not needed before ~23us -- issue them on the
        # sync ring so the exp engine's sequencer only pays issue time for
        # misc/wkq/wqk/seqball before its first activation
        wv_sb = const.tile([128, DCH, F], BF, name="wv_sb")
        nc.sync.dma_start(out=wv_sb[:], in_=wv_d.ap().rearrange("(c p) f -> p c f", p=128))
        identb_sb = const.tile([128, 128], BF, name="identb_sb")
        nc.sync.dma_start(out=identb_sb[:], in_=identb_d.ap())

        # fp8 seq (2 MB) loads FIRST: the entire score pipeline depends on it
        seqf8 = []
        for c in range(DR):
            t = big.tile([128, 2, S], F8, name=f"seqf8_{c}")
            nc.sync.dma_start(
                out=t[:],
                in_=seqf8_d[c * 256 : (c + 1) * 256, :].rearrange(
                    "(p i) s -> p i s", i=2
                ),
            )
            seqf8.append(t)
        # bf16 seq (4 MB) for the V projection: ONE DMA instruction (the ACT
        # sequencer pays ~0.7us of issue time PER dma_start -- eight of them
        # were delaying the first exp by ~5us), best transfer efficiency, and
        # the V matmuls start in one stall-free burst once it lands
        seqball = big.tile([128, DCH, S], BF, name="seqball")
        nc.scalar.dma_start(
            out=seqball[:], in_=seqb_d.ap().rearrange("(c p) s -> p c s", p=128)
        )
        seqb = [seqball[:, c, :] for c in range(DCH)]
        identf_sb = const.tile([128, 128], F32, name="identf_sb")
        nc.sync.dma_start(out=identf_sb[:], in_=identf_d.ap())

        # preload the exp table set early so the table-load DMA overlaps phase 1
        dummy_sb = const.tile([1, 1], F32, name="dummy_sb")
        nc.scalar.activation(
            out=dummy_sb[:],
            in_=misc_sb[0:1, 0:1],
            func=mybir.ActivationFunctionType.Exp,
            scale=1.0,
        )

        # kqT: k on rows 0:64 (pair A lhsT), q on rows 64:128 (pair B rhs)
        # kq2T (reversed stacking): q on rows 0:64 (pair A rhs), k on rows
        # 64:128 (pair B lhsT) -- both layouts straight from lane-aligned
        # drains, no partition-shift copies needed
        kqT_sb = big.tile([128, S], BF, name="kqT_sb")
        kq2T_sb = big.tile([128, S], BF, name="kq2T_sb")
        vT_sb = big.tile([F, S], BF, name="vT_sb")
        v_sb = big.tile([128, KCH, F + 1], BF, name="v_sb")

        bkq_ap = misc_sb[:, 0:1]  # stacked 32*[bk; bq]
        bqk_ap = misc_sb[:, 1:2]  # stacked 32*[bq; bk]
        bv_ap = misc_sb[0:F, 2:3]
        mask01 = misc_sb[:, 3:]  # [128, 16] 1.0 = keep, 0.0 = masked out

        # ones-column of ve := keep-mask (masked keys contribute 0 to the sums)
        nc.vector.tensor_copy(v_sb[:, :, F], mask01)

        out_sb = big.tile([128, S // 128, F], F32, name="out_sb")
        out_r = out_d.ap().rearrange("(c p) f -> p c f", p=128)
        ctx_tiles = {}
        pending_ctx = []  # deferred ctx matmuls: the strict-FIFO PE never
        # waits on an in-flight exp, and the backlog keeps ScalarE fed

        def emit_ctx(qc, p, expq):
            ctx_ps = ctx_tiles[qc]
            ka, kb = 2 * p, 2 * p + 1
            nc.tensor.matmul(
                ctx_ps[:], v_sb[:, ka, :], expq[:, 0:SC], start=(p == 0), stop=False
            )
            nc.tensor.matmul(
                ctx_ps[:],
                v_sb[:, kb, :],
                expq[:, SC : 2 * SC],
                start=False,
                stop=(p == KCH // 2 - 1),
            )

        def pop_ctx(n):
            for _ in range(min(n, len(pending_ctx))):
                emit_ctx(*pending_ctx.pop(0))

        def pair_block(qc, p, pops=0):
            # scores for key chunks (2p, 2p+1) x q-chunk qc, then exp.
            qsl = slice(qc * SC, (qc + 1) * SC)
            if qc not in ctx_tiles:
                ctx_tiles[qc] = ps.tile(
                    [F + 1, SC], F32, tag="ctx", bufs=1, name=f"ctx_ps{qc}"
                )
            ka, kb = 2 * p, 2 * p + 1
            ps_pair = ps.tile(
                [128, 2 * SC], F32, tag="pair", bufs=2, name=f"ps_pair_{qc}_{p}"
            )
            # chunk A on array rows 0:64, chunk B on rows 64:128 --
            # disjoint row groups run concurrently on the PE
            nc.tensor.matmul(
                ps_pair[:, 0:SC],
                kqT_sb[0:F, ka * 128 : (ka + 1) * 128],
                kq2T_sb[0:F, qsl],
                start=True,
                stop=True,
            )
            nc.tensor.matmul(
                ps_pair[:, SC : 2 * SC],
                kq2T_sb[64:128, kb * 128 : (kb + 1) * 128],
                kqT_sb[64:128, qsl],
                start=True,
                stop=True,
            )
            expq = sbw.tile(
                [128, 2 * SC], BF, tag="expq", bufs=18, name=f"expq_{qc}_{p}"
            )
            nc.scalar.activation(
                out=expq[:],
                in_=ps_pair[:],
                func=mybir.ActivationFunctionType.Exp,
                scale=SCALE,
            )
            pending_ctx.append((qc, p, expq))
            pop_ctx(pops)

        def finalize(qc):
            # drain ctx; reciprocal the sumexp row ONCE, then every transpose
            # carries 1/sum in column 64 and the normalize is one multiply
            ctx_ps = ctx_tiles.pop(qc)
            ctxTq = sbw.tile([F + 1, SC], F32, tag="ctxTq", bufs=2, name=f"ctxTq{qc}")
            nc.vector.tensor_copy(ctxTq[:], ctx_ps[:])
            nc.vector.reciprocal(ctxTq[F : F + 1, :], ctxTq[F : F + 1, :])
            for i in range(SC // 128):
                t = qc * 4 + i
                ctp = ps.tile([128, F + 1], F32, tag="pskq", bufs=2, name=f"ctp{t}")
                nc.tensor.transpose(
                    ctp[:],
                    ctxTq[:, i * 128 : (i + 1) * 128],
                    identf_sb[0 : F + 1, 0 : F + 1],
                )
                nc.vector.tensor_scalar_mul(
                    out_sb[:, t, :], ctp[:, 0:F], ctp[:, F : F + 1]
                )
            nc.sync.dma_start(
                out=out_r[:, qc * 4 : (qc + 1) * 4, :],
                in_=out_sb[:, qc * 4 : (qc + 1) * 4, :],
            )

        # ---- K/Q projections (fp8 DoubleRow, 256-deep contraction chunks),
        # with all 16 q-chunk-0/1 score pair-blocks emitted as deps arrive ----
        for sj in range(S // SC):
            ps_kq = ps.tile([128, SC], F32, tag="pskq", bufs=2, name=f"ps_kq{sj}")
            ps_kq2 = ps.tile([128, SC], F32, tag="pskq", bufs=2, name=f"ps_kq2_{sj}")
            for c in range(DR):
                rhs = seqf8[c][:, :, sj * SC : (sj + 1) * SC]
                st = dict(start=(c == 0), stop=(c == DR - 1))
                nc.tensor.matmul(
                    ps_kq[:], wkq_sb[:, c, :, :], rhs,
                    perf_mode=mybir.MatmulPerfMode.DoubleRow, **st
                )
                nc.tensor.matmul(
                    ps_kq2[:], wqk_sb[:, c, :, :], rhs,
                    perf_mode=mybir.MatmulPerfMode.DoubleRow, **st
                )
            sl = slice(sj * SC, (sj + 1) * SC)
            last_kq_drain = nc.vector.tensor_scalar_add(kqT_sb[:, sl], ps_kq[:], bkq_ap)
            last_kq_drain = nc.vector.tensor_scalar_add(kq2T_sb[:, sl], ps_kq2[:], bqk_ap)

            # wavefront: pair(qc, p) scores need kT of s-chunk p//2, qT of qc
            if sj == 0:
                for p in (0, 1):
                    pair_block(0, p)
            elif sj == 1:
                for p in (0, 1):
                    pair_block(1, p)
                for p in (2, 3):
                    pair_block(0, p)
            elif sj == 2:
                for p in (2, 3):
                    pair_block(1, p)
                for p in (4, 5):
                    pair_block(0, p)
            else:
                for p in (4, 5):
                    pair_block(1, p)
                for p in (6, 7):
                    pair_block(0, p)
                    pair_block(1, p)

        # ---- V projection (bf16) + v transposes; ScalarE chews the exp
        # backlog while this runs ----
        for sj in range(S // SC):
            ps_v = ps.tile([F, SC], F32, tag="psv", bufs=1, name=f"ps_v{sj}")  # shares tag with vtp
            for c in range(DCH):
                nc.tensor.matmul(
                    ps_v[:],
                    wv_sb[:, c, :],
                    seqb[c][:, sj * SC : (sj + 1) * SC],
                    start=(c == 0),
                    stop=(c == DCH - 1),
                )
            sl = slice(sj * SC, (sj + 1) * SC)
            nc.vector.tensor_scalar_add(vT_sb[:, sl], ps_v[:], bv_ap)
            # transpose into natural [k, f] layout, zeroing masked keys' V rows
            for t in range(sj * 4, sj * 4 + 4):
                vtp = ps.tile([128, F], BF, tag="psv", bufs=1, name=f"vtp{t}")
                nc.tensor.transpose(
                    vtp[:],
                    vT_sb[:, t * 128 : (t + 1) * 128],
                    identb_sb[0:F, 0:F],
                )
                nc.vector.tensor_scalar_mul(
                    v_sb[:, t, 0:F], vtp[:], mask01[:, t : t + 1]
                )

        # ---- remaining q-chunks; each block retires two deferred ctx matmuls ----
        for p in range(KCH // 2):
            pair_block(2, p, pops=2)
        # all q-chunk 0/1 ctx matmuls have been retired; finalize both BEFORE
        # any q-chunk 2/3 ctx matmul reuses their psum slots
        finalize(0)
        finalize(1)
        for p in range(KCH // 2):
            pair_block(3, p, pops=4)
        pop_ctx(len(pending_ctx))
        finalize(2)
        finalize(3)


_NC_CACHE = None


def _get_nc():
    global _NC_CACHE
    if _NC_CACHE is None:
        nc = bacc.Bacc("TRN2", target_bir_lowering=False, debug=False)
        _emit(nc)
        _NC_CACHE = nc
    return _NC_CACHE


def make_in_maps(seq, mask, Wq, bq, Wk, bk, Wv, bv):
    bf16 = ml_dtypes.bfloat16
    f8 = ml_dtypes.float8_e4m3
    seq = np.asarray(seq, dtype=np.float32)
    mask = np.asarray(mask).astype(bool)
    wkq = np.concatenate(
        [np.asarray(Wk, dtype=np.float32), np.asarray(Wq, dtype=np.float32)], axis=1
    )  # [D, 128]
    wkq_h = np.ascontiguousarray(wkq * 32.0).astype(f8)
    wqk = np.concatenate(
        [np.asarray(Wq, dtype=np.float32), np.asarray(Wk, dtype=np.float32)], axis=1
    )
    wqk_h = np.ascontiguousarray(wqk * 32.0).astype(f8)
    wv_h = np.ascontiguousarray(np.asarray(Wv, dtype=np.float32)).astype(bf16)
    identb = np.eye(128, dtype=bf16)
    identf = np.eye(128, dtype=np.float32)
    in_maps = []
    for b in range(NCORES):
        seqT = np.ascontiguousarray(seq[b].T)  # [D, S] f32
        misc = np.zeros((128, 3 + KCH), dtype=np.float32)
        misc[0:F, 0] = 32.0 * np.asarray(bk, dtype=np.float32)
        misc[64:128, 0] = 32.0 * np.asarray(bq, dtype=np.float32)
        misc[0:F, 1] = 32.0 * np.asarray(bq, dtype=np.float32)
        misc[64:128, 1] = 32.0 * np.asarray(bk, dtype=np.float32)
        misc[0:F, 2] = np.asarray(bv, dtype=np.float32)
        # keep-mask: misc[p, 3+c] = 0.0 if key c*128+p is masked out else 1.0
        misc[:, 3:] = np.where(mask[b], np.float32(0.0), np.float32(1.0)).reshape(
            KCH, 128
        ).T
        in_maps.append(
            {
                "seqf8": seqT.astype(f8),
                "seqb": seqT.astype(bf16),
                "wkq": wkq_h,
                "wqk": wqk_h,
                "wv": wv_h,
                "misc": misc,
                "identb": identb,
                "identf": identf,
            }
        )
    return in_maps


def run(in_maps, trace=False, **kw):
    nc = _get_nc()
    return run_bass_kernel_spmd(
        nc, in_maps, core_ids=list(range(NCORES)), trace=trace, **kw
    )


def kernel(seq, mask, Wq, bq, Wk, bk, Wv, bv):
    in_maps = make_in_maps(seq, mask, Wq, bq, Wk, bk, Wv, bv)
    res = run(in_maps)
    out = np.stack(
        [np.asarray(res.results[i]["out"], dtype=np.float32) for i in range(NCORES)],
        axis=0,
    )
    return out
